# revision 28
# baseline (speedup 1.0000x reference)
"""CfC block (LayerNorm -> Linear -> GRU scan -> Linear + residual) on 8 trn2 cores.

Strategy: data-parallel over batch (16 samples -> 2 per core), weights replicated,
no collectives. Everything after LayerNorm runs in transposed (feature-on-partition)
layout so the sequential GRU scan needs no per-step transposes.
"""

import sys

for _p in ("/opt/trn_rl_repo",):
    if _p not in sys.path:
        sys.path.insert(0, _p)

from contextlib import ExitStack

import ml_dtypes
import numpy as np

import concourse.bass as bass
import concourse.mybir as mybir
import concourse.tile as tile
from concourse.bass import ds, ts
from concourse.bass_utils import run_bass_kernel_spmd
from concourse.masks import make_identity
from concourse.tile_rust import add_dep_helper

F32 = mybir.dt.float32
BF16 = mybir.dt.bfloat16
AF = mybir.ActivationFunctionType

B, S_FULL, D, U = 16, 1024, 1024, 512
G = 3 * U
N_CORES = 8
B_LOC = B // N_CORES  # 2 samples per core
LN_EPS = 1e-5

KC_D = D // 128  # 8  k-chunks for D contraction
KC_U = U // 128  # 4  k-chunks for U contraction
MT_U = U // 128  # 4  u output tiles
MT_G = G // 128  # 12 gate output tiles (0-3 r, 4-7 z, 8-11 n)


def build_program(S: int, debug: bool = False):
    """One SPMD Bass program; every core runs it on its own batch shard.

    Token order everywhere is time-major: tok = t*B_LOC + j.
    """
    T = S * B_LOC
    TT = T // 128  # 128-token tiles
    NCOL = min(512, T)  # token-chunk width for GEMM moving operands
    NT = T // NCOL
    C_PER = max(1, S // TT)  # scan steps between out-proj tile emissions

    nc = bass.Bass()

    x_ext = nc.declare_dram_parameter("x", [B_LOC, S, D], F32, isOutput=False)
    xres_ext = nc.declare_dram_parameter("xres", [B_LOC, S, D], F32, isOutput=False)
    lngT_ext = nc.declare_dram_parameter("lngT", [128, KC_D], F32, isOutput=False)
    lnbT_ext = nc.declare_dram_parameter("lnbT", [128, KC_D], F32, isOutput=False)
    w_inT_ext = nc.declare_dram_parameter("w_inT", [128, KC_D, U], BF16, isOutput=False)
    binT_ext = nc.declare_dram_parameter("binT", [128, MT_U], F32, isOutput=False)
    w_ihT_ext = nc.declare_dram_parameter("w_ihT", [128, KC_U, G], BF16, isOutput=False)
    bgT_ext = nc.declare_dram_parameter("bgT", [128, MT_G], F32, isOutput=False)
    w_hhT_ext = nc.declare_dram_parameter("w_hhT", [128, KC_U, G], BF16, isOutput=False)
    bhhnT_ext = nc.declare_dram_parameter("bhhnT", [128, 2 * KC_U], F32, isOutput=False)
    w_outT_ext = nc.declare_dram_parameter("w_outT", [128, KC_U, D], BF16, isOutput=False)

    out_ext = nc.declare_dram_parameter("out", [B_LOC, S, D], F32, isOutput=True)
    hlastT_ext = nc.declare_dram_parameter("hlastT", [128, 2 * KC_U], F32, isOutput=True)
    if debug:
        xnT_d = nc.declare_dram_parameter("xnT_d", [128, KC_D, S * B_LOC], BF16, isOutput=True)
        uT_d = nc.declare_dram_parameter("uT_d", [128, MT_U, S * B_LOC], BF16, isOutput=True)
        gxrz_d = nc.declare_dram_parameter("gxrz_d", [128, 2 * KC_U, S * B_LOC], BF16, isOutput=True)
        gxn_d = nc.declare_dram_parameter("gxn_d", [128, KC_U, S * B_LOC], F32, isOutput=True)
        yT_d = nc.declare_dram_parameter("yT_d", [128, KC_U, S * B_LOC], BF16, isOutput=True)

    # time-major views of the [j, t, d] DRAM tensors: [t, j, d]
    x_tm = x_ext[:, :, :].rearrange("j t d -> t j d")
    xres_tm = xres_ext[:, :, :].rearrange("j t d -> t j d")
    out_tm = out_ext[:, :, :].rearrange("j t d -> t j d")

    with tile.TileContext(nc) as tc, ExitStack() as ctx:
        persist = ctx.enter_context(tc.tile_pool(name="persist", bufs=1))
        ln_pool = ctx.enter_context(tc.tile_pool(name="ln", bufs=2))
        ln_small = ctx.enter_context(tc.tile_pool(name="ln_small", bufs=4))
        tr_psum = ctx.enter_context(tc.tile_pool(name="tr_psum", bufs=2, space="PSUM"))
        mm_psum = ctx.enter_context(tc.tile_pool(name="mm_psum", bufs=2, space="PSUM"))
        sc_psum = ctx.enter_context(tc.tile_pool(name="sc_psum", bufs=2, space="PSUM"))
        c_psum = ctx.enter_context(tc.tile_pool(name="c_psum", bufs=2, space="PSUM"))
        ew_pool = ctx.enter_context(tc.tile_pool(name="ew", bufs=3))
        h_pool = ctx.enter_context(tc.tile_pool(name="h", bufs=3))
        c_pool = ctx.enter_context(tc.tile_pool(name="c", bufs=2))

        # ---- persistent SBUF tensors ----
        xnT = persist.tile([128, KC_D, T], BF16)  # normalized input, transposed
        uT = persist.tile([128, MT_U, T], BF16)  # in-proj output, transposed
        gxrzT = persist.tile([128, 2 * KC_U, T], BF16)  # r,z input gates (incl biases)
        gxnT = persist.tile([128, KC_U, T], F32)  # n input gate (incl b_ih part)
        yT = persist.tile([128, KC_U, T], BF16)  # hidden states (scan output)

        w_inT_sb = persist.tile([128, KC_D, U], BF16)
        w_ihT_sb = persist.tile([128, KC_U, G], BF16)
        w_hhT_sb = persist.tile([128, KC_U, G], BF16)
        w_outT_sb = persist.tile([128, KC_U, D], BF16)
        lngT_sb = persist.tile([128, KC_D], F32)
        lnbT_sb = persist.tile([128, KC_D], F32)
        binT_sb = persist.tile([128, MT_U], F32)
        bgT_sb = persist.tile([128, MT_G], F32)
        bhhnT_sb = persist.tile([128, 2 * KC_U], F32)
        ident = persist.tile([128, 128], F32)
        eps_t = persist.tile([128, 1], F32)
        h0bf = persist.tile([128, 2 * KC_U], BF16)
        h0f = persist.tile([128, 2 * KC_U], F32)

        nc.sync.dma_start(out=w_inT_sb, in_=w_inT_ext[:, :, :])
        nc.sync.dma_start(out=w_ihT_sb, in_=w_ihT_ext[:, :, :])
        nc.sync.dma_start(out=w_hhT_sb, in_=w_hhT_ext[:, :, :])
        nc.sync.dma_start(out=w_outT_sb, in_=w_outT_ext[:, :, :])
        nc.sync.dma_start(out=lngT_sb, in_=lngT_ext[:, :])
        nc.sync.dma_start(out=lnbT_sb, in_=lnbT_ext[:, :])
        nc.sync.dma_start(out=binT_sb, in_=binT_ext[:, :])
        nc.sync.dma_start(out=bgT_sb, in_=bgT_ext[:, :])
        nc.sync.dma_start(out=bhhnT_sb, in_=bhhnT_ext[:, :])
        make_identity(nc, ident)
        nc.vector.memset(eps_t, LN_EPS)
        nc.vector.memset(h0bf, 0.0)
        nc.vector.memset(h0f, 0.0)
        # DVE "touch" of DMA-delivered tensors consumed by DVE ops, so their
        # queue sems are observed once here instead of adding waits later.
        touch = persist.tile([128, 1], F32)
        for src in (lngT_sb, lnbT_sb, binT_sb, bgT_sb, bhhnT_sb):
            nc.vector.tensor_copy(touch, src[:, 0:1])
        # First PE instruction: absorb the gpsimd identity wait.
        pt0 = tr_psum.tile([128, 128], F32, tag="pt")
        nc.tensor.transpose(pt0, ident, ident)

        # ---- Phase 1: LayerNorm (natural layout) + transpose to xnT ----
        for i in range(TT):
            x_t = ln_pool.tile([128, D], F32, tag="x_t")
            nc.sync.dma_start(
                out=x_t,
                in_=x_tm[i * (128 // B_LOC) : (i + 1) * (128 // B_LOC), :, :],
            )
            stats = ln_small.tile([128, 2, 6], F32, tag="stats")
            xv = x_t.rearrange("p (h q) -> p h q", h=2)
            nc.vector.bn_stats(out=stats[:, 0, :], in_=xv[:, 0, :])
            nc.vector.bn_stats(out=stats[:, 1, :], in_=xv[:, 1, :])
            mv = ln_small.tile([128, 2], F32, tag="mv")
            nc.vector.bn_aggr(out=mv, in_=stats)
            rstd = ln_small.tile([128, 1], F32, tag="rstd")
            nc.scalar.activation(
                out=rstd, in_=mv[:, 1:2], func=AF.Sqrt, bias=eps_t, scale=1.0
            )
            nc.vector.reciprocal(out=rstd, in_=rstd)
            xc_t = ln_pool.tile([128, D], F32, tag="xc_t")
            nc.vector.tensor_scalar(
                out=xc_t,
                in0=x_t,
                scalar1=mv[:, 0:1],
                scalar2=rstd,
                op0=mybir.AluOpType.subtract,
                op1=mybir.AluOpType.mult,
            )
            for dc in range(KC_D):
                pt = tr_psum.tile([128, 128], F32, tag="pt")
                nc.tensor.transpose(pt, xc_t[:, ts(dc, 128)], ident)
                nc.vector.tensor_scalar(
                    out=xnT[:, dc, ts(i, 128)],
                    in0=pt,
                    scalar1=lngT_sb[:, dc : dc + 1],
                    scalar2=lnbT_sb[:, dc : dc + 1],
                    op0=mybir.AluOpType.mult,
                    op1=mybir.AluOpType.add,
                )
            if i == 0:
                # Dummy matmuls: let PE observe each weight tensor's DMA queue
                # sem once, keeping every real matmul at <=1 sync wait.
                pdum = mm_psum.tile([128, NCOL], F32, tag="pmm")
                for wsb in (w_inT_sb, w_ihT_sb, w_hhT_sb, w_outT_sb):
                    nc.tensor.matmul(
                        pdum[:, 0:2],
                        lhsT=wsb[:, 0, 0:128],
                        rhs=h0bf[:, 0:2],
                        start=True,
                        stop=True,
                        skip_group_check=True,
                    )

        # ---- Phase 2: in-proj  uT = w_in @ xn^T  (+ b_in) ----
        for m in range(MT_U):
            for ncol in range(NT):
                pa = mm_psum.tile([128, NCOL], F32, tag="pmm")
                for kc in range(KC_D):
                    nc.tensor.matmul(
                        pa,
                        lhsT=w_inT_sb[:, kc, ts(m, 128)],
                        rhs=xnT[:, kc, ts(ncol, NCOL)],
                        start=(kc == 0),
                        stop=(kc == KC_D - 1),
                    )
                nc.vector.tensor_scalar_add(
                    out=uT[:, m, ts(ncol, NCOL)],
                    in0=pa,
                    scalar1=binT_sb[:, m : m + 1],
                )

        # ---- Phase 3: input gates  gx^T = w_ih @ u^T (+ biases) ----
        for m in range(MT_G):
            for ncol in range(NT):
                pb = mm_psum.tile([128, NCOL], F32, tag="pmm")
                for kc in range(KC_U):
                    nc.tensor.matmul(
                        pb,
                        lhsT=w_ihT_sb[:, kc, ts(m, 128)],
                        rhs=uT[:, kc, ts(ncol, NCOL)],
                        start=(kc == 0),
                        stop=(kc == KC_U - 1),
                    )
                if m < 2 * KC_U:
                    dst = gxrzT[:, m, ts(ncol, NCOL)]
                else:
                    dst = gxnT[:, m - 2 * KC_U, ts(ncol, NCOL)]
                nc.vector.tensor_scalar_add(
                    out=dst, in0=pb, scalar1=bgT_sb[:, m : m + 1]
                )

        # ---- Phase 4: GRU scan (out-proj tiles interleaved into PE idle gaps) ----
        h_prev = h0f
        n_c_emitted = 0
        prev_pe = [None]  # last emitted scan matmul, to pin openers in place

        def pin(op):
            if prev_pe[0] is not None:
                add_dep_helper(op.ins, prev_pe[0].ins, False, "pin opener order")
            return op

        def emit_outproj_tile(mt):
            xr_t = c_pool.tile([128, D], F32, tag="xr_t")
            nc.sync.dma_start(
                out=xr_t,
                in_=xres_tm[mt * (128 // B_LOC) : (mt + 1) * (128 // B_LOC), :, :],
            )
            o_t = c_pool.tile([128, D], F32, tag="o_t")
            for dn in range(D // 512):
                pc = c_psum.tile([128, 512], F32, tag="pc")
                pin(nc.tensor.matmul(
                    pc[:, 0:2],
                    lhsT=w_outT_sb[:, 0, 0:128],
                    rhs=h0bf[:, 0:2],
                    start=True,
                    stop=True,
                    skip_group_check=True,
                ))
                for kc in range(KC_U):
                    nc.tensor.matmul(
                        pc,
                        lhsT=yT[:, kc, ts(mt, 128)],
                        rhs=w_outT_sb[:, kc, ts(dn, 512)],
                        start=(kc == 0),
                        stop=(kc == KC_U - 1),
                    )
                nc.vector.tensor_add(o_t[:, ts(dn, 512)], pc, xr_t[:, ts(dn, 512)])
            nc.sync.dma_start(
                out=out_tm[mt * (128 // B_LOC) : (mt + 1) * (128 // B_LOC), :, :],
                in_=o_t,
            )

        for t in range(S):
            P = sc_psum.tile([128, 512], F32, tag="P")  # full bank: isolate slots
            # opener: absorbs the PSUM bank-reuse wait with observed inputs
            pin(nc.tensor.matmul(
                P[:, 0:B_LOC],
                lhsT=w_hhT_sb[:, 0, 0:128],
                rhs=h0bf[:, 0:B_LOC],
                start=True,
                stop=True,
                skip_group_check=True,
            ))
            for m in range(MT_G):
                for kc in range(KC_U):
                    if t == 0:
                        rhs = h0bf[:, ts(kc, B_LOC)]
                    else:
                        rhs = yT[:, kc, ds(B_LOC * (t - 1), B_LOC)]
                    prev_pe[0] = nc.tensor.matmul(
                        P[:, ts(m, B_LOC)],
                        lhsT=w_hhT_sb[:, kc, ts(m, 128)],
                        rhs=rhs,
                        start=(kc == 0),
                        stop=(kc == KC_U - 1),
                    )

            # elementwise GRU cell, all [128, 8] fp32 in transposed layout
            rz_pre = ew_pool.tile([128, 4 * KC_U], F32, tag="rz_pre")
            nc.vector.tensor_add(
                rz_pre.rearrange("p (m j) -> p m j", j=B_LOC),
                P[:, 0 : 4 * KC_U].rearrange("p (m j) -> p m j", j=B_LOC),
                gxrzT[:, :, ds(B_LOC * t, B_LOC)],
            )
            rz = ew_pool.tile([128, 4 * KC_U], F32, tag="rz")
            nc.scalar.activation(out=rz, in_=rz_pre, func=AF.Sigmoid)
            hnb = ew_pool.tile([128, 2 * KC_U], F32, tag="hnb")
            nc.vector.tensor_add(hnb, P[:, 4 * KC_U : 6 * KC_U], bhhnT_sb)
            nr = ew_pool.tile([128, 2 * KC_U], F32, tag="nr")
            nc.vector.tensor_mul(nr, rz[:, 0 : 2 * KC_U], hnb)
            npre = ew_pool.tile([128, 2 * KC_U], F32, tag="npre")
            nc.vector.tensor_add(
                npre.rearrange("p (m j) -> p m j", j=B_LOC),
                nr.rearrange("p (m j) -> p m j", j=B_LOC),
                gxnT[:, :, ds(B_LOC * t, B_LOC)],
            )
            n_t = ew_pool.tile([128, 2 * KC_U], F32, tag="n_t")
            nc.scalar.activation(out=n_t, in_=npre, func=AF.Tanh)
            d_t = ew_pool.tile([128, 2 * KC_U], F32, tag="d_t")
            nc.vector.tensor_sub(d_t, h_prev, n_t)
            e_t = ew_pool.tile([128, 2 * KC_U], F32, tag="e_t")
            nc.vector.tensor_mul(e_t, rz[:, 2 * KC_U : 4 * KC_U], d_t)
            h_new = h_pool.tile([128, 2 * KC_U], F32, tag="h_new")
            nc.vector.tensor_add(h_new, n_t, e_t)
            nc.vector.tensor_copy(
                yT[:, :, ds(B_LOC * t, B_LOC)],
                h_new.rearrange("p (m j) -> p m j", j=B_LOC),
            )
            h_prev = h_new

            # slot an out-proj tile into the PE gap once its y columns exist
            if (t + 1) % C_PER == 0:
                mt = (t + 1) // C_PER - 1
                if mt < TT:
                    emit_outproj_tile(mt)
                    n_c_emitted += 1

        for mt in range(n_c_emitted, TT):
            emit_outproj_tile(mt)

        nc.sync.dma_start(out=hlastT_ext[:, :], in_=h_prev)
        if debug:
            nc.sync.dma_start(out=xnT_d[:, :, :], in_=xnT)
            nc.sync.dma_start(out=uT_d[:, :, :], in_=uT)
            nc.sync.dma_start(out=gxrz_d[:, :, :], in_=gxrzT)
            nc.sync.dma_start(out=gxn_d[:, :, :], in_=gxnT)
            nc.sync.dma_start(out=yT_d[:, :, :], in_=yT)

    _legalize_waits(nc)
    return nc


# Sequencer-class instructions accept many embedded sync waits; engine-datapath
# instructions accept only ONE (walrus "Too many sync wait commands"). Move
# excess waits onto a preceding same-engine sequencer NOP.
_SEQ_CLASS = {
    "InstNoOp", "InstDrain", "InstEventSemaphore", "InstUnconditionalBranch",
    "InstConditionalBranch", "InstRegisterMove", "InstCall", "InstISA",
    "InstRegisterAlu", "InstCompareAndBranch",
}


def _legalize_waits(nc):
    n_split = 0
    for fn in nc.m.functions:
        for blk in fn.blocks:
            il = blk.instructions
            idx = 0
            while idx < len(il):
                inst = il[idx]
                si = inst.sync_info
                if si is not None and si.on_wait and len(si.on_wait) > 1:
                    waits = list(si.on_wait)
                    keep = waits[-1:]
                    rest = waits[:-1]
                    nops = []
                    for w in rest:
                        nop = mybir.InstNoOp(
                            name=f"legw-{n_split}",
                            engine=inst.engine,
                            ins=[],
                            outs=[],
                        )
                        nop.sync_info = mybir.SyncInfo(on_wait=[w], on_update=[])
                        nops.append(nop)
                        n_split += 1
                    inst.sync_info = mybir.SyncInfo(
                        on_wait=keep, on_update=list(si.on_update or [])
                    )
                    for j, nop in enumerate(nops):
                        il.insert(idx + j, nop)
                    idx += len(nops)
                idx += 1
    return n_split


def _prep_shared_inputs(ln_gamma, ln_beta, w_in, b_in, w_ih, w_hh, b_ih, b_hh, w_out):
    """Host-side weight layout transforms (shared across cores)."""
    bf = ml_dtypes.bfloat16

    def chunked_T(w):  # [O, I] -> [128, I//128, O] (transposed, k-chunked)
        o, i = w.shape
        return np.ascontiguousarray(
            w.T.reshape(i // 128, 128, o).transpose(1, 0, 2)
        )

    lngT = np.ascontiguousarray(ln_gamma.reshape(KC_D, 128).T).astype(np.float32)
    lnbT = np.ascontiguousarray(ln_beta.reshape(KC_D, 128).T).astype(np.float32)
    w_inT = chunked_T(w_in).astype(bf)
    binT = np.ascontiguousarray(b_in.reshape(MT_U, 128).T).astype(np.float32)
    w_ihT = chunked_T(w_ih).astype(bf)
    # r,z gate tiles carry b_ih+b_hh; n tiles carry only b_ih (b_hh_n applied in-scan)
    bg = (b_ih + b_hh).astype(np.float32).copy()
    bg[2 * U :] = b_ih[2 * U :]
    bgT = np.ascontiguousarray(bg.reshape(MT_G, 128).T).astype(np.float32)
    w_hhT = chunked_T(w_hh).astype(bf)
    bhhn = b_hh[2 * U :].astype(np.float32).reshape(KC_U, 128).T  # [128, 4]
    bhhnT = np.ascontiguousarray(
        np.repeat(bhhn[:, :, None], B_LOC, axis=2).reshape(128, KC_U * B_LOC)
    )
    w_outT = chunked_T(w_out).astype(bf)
    return dict(
        lngT=lngT, lnbT=lnbT, w_inT=w_inT, binT=binT, w_ihT=w_ihT,
        bgT=bgT, w_hhT=w_hhT, bhhnT=bhhnT, w_outT=w_outT,
    )


_BUILD_CACHE = {}


def _get_program(S):
    if S not in _BUILD_CACHE:
        _BUILD_CACHE[S] = build_program(S)
    return _BUILD_CACHE[S]


LAST_EXEC_TIME_NS = None


def run(x, ln_gamma, ln_beta, w_in, b_in, w_ih, w_hh, b_ih, b_hh, w_out, b_out,
        trace=False, S=S_FULL):
    global LAST_EXEC_TIME_NS
    x = np.asarray(x, dtype=np.float32)
    b_full = x.shape[0]
    n_cores = b_full // B_LOC
    shared = _prep_shared_inputs(
        np.asarray(ln_gamma, np.float32), np.asarray(ln_beta, np.float32),
        np.asarray(w_in, np.float32), np.asarray(b_in, np.float32),
        np.asarray(w_ih, np.float32), np.asarray(w_hh, np.float32),
        np.asarray(b_ih, np.float32), np.asarray(b_hh, np.float32),
        np.asarray(w_out, np.float32),
    )
    xres = x + np.asarray(b_out, np.float32)[None, None, :]

    nc = _get_program(S)
    in_maps = []
    for k in range(n_cores):
        m = dict(shared)
        m["x"] = np.ascontiguousarray(x[k * B_LOC : (k + 1) * B_LOC])
        m["xres"] = np.ascontiguousarray(xres[k * B_LOC : (k + 1) * B_LOC])
        in_maps.append(m)

    try:
        res = run_bass_kernel_spmd(nc, in_maps, list(range(n_cores)), trace=trace)
    except ModuleNotFoundError:
        res = run_bass_kernel_spmd(nc, in_maps, list(range(n_cores)), trace=False)
    LAST_EXEC_TIME_NS = res.exec_time_ns

    out = np.empty((b_full, S, D), np.float32)
    h_last = np.empty((b_full, U), np.float32)
    for k in range(n_cores):
        out[k * B_LOC : (k + 1) * B_LOC] = res.results[k]["out"]
        hlT = res.results[k]["hlastT"].reshape(128, KC_U, B_LOC)
        h_last[k * B_LOC : (k + 1) * B_LOC] = (
            hlT.transpose(2, 1, 0).reshape(B_LOC, U)
        )
    return out, h_last


def kernel(x, ln_gamma, ln_beta, w_in, b_in, w_ih, w_hh, b_ih, b_hh, w_out, b_out):
    return run(
        x, ln_gamma, ln_beta, w_in, b_in, w_ih, w_hh, b_ih, b_hh, w_out, b_out,
        trace=False, S=S_FULL,
    )


# revision 44
# speedup vs baseline: 1.3070x; 1.3070x over previous
"""CfC block (LayerNorm -> Linear -> GRU scan -> Linear + residual) on 8 trn2 cores.

Strategy: data-parallel over batch (16 samples -> 2 per core), weights replicated,
no collectives. Everything after LayerNorm runs in transposed (feature-on-partition)
layout so the sequential GRU scan needs no per-step transposes.
"""

import sys

for _p in ("/opt/trn_rl_repo",):
    if _p not in sys.path:
        sys.path.insert(0, _p)

from contextlib import ExitStack

import ml_dtypes
import numpy as np

import concourse.bass as bass
import concourse.mybir as mybir
import concourse.tile as tile
from concourse.bass import ds, ts
from concourse.bass_utils import run_bass_kernel_spmd
from concourse.masks import make_identity
from concourse.tile_rust import add_dep_helper

F32 = mybir.dt.float32
BF16 = mybir.dt.bfloat16
AF = mybir.ActivationFunctionType

B, S_FULL, D, U = 16, 1024, 1024, 512
G = 3 * U
N_CORES = 8
B_LOC = B // N_CORES  # 2 samples per core
LN_EPS = 1e-5

KC_D = D // 128  # 8  k-chunks for D contraction
KC_U = U // 128  # 4  k-chunks for U contraction
MT_U = U // 128  # 4  u output tiles
MT_G = G // 128  # 12 gate output tiles (0-3 r, 4-7 z, 8-11 n)


def build_program(S: int, debug: bool = False):
    """One SPMD Bass program; every core runs it on its own batch shard.

    Token order everywhere is time-major: tok = t*B_LOC + j.
    """
    T = S * B_LOC
    TT = T // 128  # 128-token tiles
    NCOL = min(512, T)  # token-chunk width for GEMM moving operands
    NT = T // NCOL
    C_PER = max(1, S // TT)  # scan steps between out-proj tile emissions

    nc = bass.Bass()

    x_ext = nc.declare_dram_parameter("x", [B_LOC, S, D], F32, isOutput=False)
    xres_ext = nc.declare_dram_parameter("xres", [B_LOC, S, D], F32, isOutput=False)
    lngT_ext = nc.declare_dram_parameter("lngT", [128, KC_D], F32, isOutput=False)
    lnbT_ext = nc.declare_dram_parameter("lnbT", [128, KC_D], F32, isOutput=False)
    w_inT_ext = nc.declare_dram_parameter("w_inT", [128, KC_D, U], BF16, isOutput=False)
    binT_ext = nc.declare_dram_parameter("binT", [128, MT_U], F32, isOutput=False)
    w_ihT_ext = nc.declare_dram_parameter("w_ihT", [128, KC_U, G], BF16, isOutput=False)
    bgT_ext = nc.declare_dram_parameter("bgT", [128, MT_G], F32, isOutput=False)
    w_hhT_ext = nc.declare_dram_parameter("w_hhT", [128, KC_U, G], BF16, isOutput=False)
    bhhnT_ext = nc.declare_dram_parameter("bhhnT", [128, 2 * KC_U], BF16, isOutput=False)
    w_outT_ext = nc.declare_dram_parameter("w_outT", [128, KC_U, D], BF16, isOutput=False)

    out_ext = nc.declare_dram_parameter("out", [B_LOC, S, D], F32, isOutput=True)
    hlastT_ext = nc.declare_dram_parameter("hlastT", [128, 2 * KC_U], F32, isOutput=True)
    if debug:
        xnT_d = nc.declare_dram_parameter("xnT_d", [128, KC_D, S * B_LOC], BF16, isOutput=True)
        uT_d = nc.declare_dram_parameter("uT_d", [128, MT_U, S * B_LOC], BF16, isOutput=True)
        gxrz_d = nc.declare_dram_parameter("gxrz_d", [128, 2 * KC_U, S * B_LOC], BF16, isOutput=True)
        gxn_d = nc.declare_dram_parameter("gxn_d", [128, KC_U, S * B_LOC], BF16, isOutput=True)
        yT_d = nc.declare_dram_parameter("yT_d", [128, KC_U, S * B_LOC], BF16, isOutput=True)

    # time-major views of the [j, t, d] DRAM tensors: [t, j, d]
    x_tm = x_ext[:, :, :].rearrange("j t d -> t j d")
    xres_tm = xres_ext[:, :, :].rearrange("j t d -> t j d")
    out_tm = out_ext[:, :, :].rearrange("j t d -> t j d")

    with tile.TileContext(nc) as tc, ExitStack() as ctx:
        persist = ctx.enter_context(tc.tile_pool(name="persist", bufs=1))
        ln_pool = ctx.enter_context(tc.tile_pool(name="ln", bufs=2))
        ln_small = ctx.enter_context(tc.tile_pool(name="ln_small", bufs=4))
        tr_psum = ctx.enter_context(tc.tile_pool(name="tr_psum", bufs=2, space="PSUM"))
        mm_psum = ctx.enter_context(tc.tile_pool(name="mm_psum", bufs=2, space="PSUM"))
        sc_psum = ctx.enter_context(tc.tile_pool(name="sc_psum", bufs=2, space="PSUM"))
        c_psum = ctx.enter_context(tc.tile_pool(name="c_psum", bufs=2, space="PSUM"))
        ew_pool = ctx.enter_context(tc.tile_pool(name="ew", bufs=3))
        h_pool = ctx.enter_context(tc.tile_pool(name="h", bufs=3))
        c_pool = ctx.enter_context(tc.tile_pool(name="c", bufs=2))

        # ---- persistent SBUF tensors ----
        xnT = persist.tile([128, KC_D, T], BF16)  # normalized input, transposed
        uT = persist.tile([128, MT_U, T], BF16)  # in-proj output, transposed
        # rows 0-7: r,z input gates (incl biases); rows 8-11: b_hh n-part
        # broadcast across all steps so one DVE add evacuates the whole PSUM.
        gxall = persist.tile([128, MT_G, T], BF16)
        gxnT = persist.tile([128, KC_U, T], BF16)  # n input gate (incl b_ih part)
        yT = persist.tile([128, KC_U, T], BF16)  # hidden states (bf16 state)

        w_inT_sb = persist.tile([128, KC_D, U], BF16)
        w_ihT_sb = persist.tile([128, KC_U, G], BF16)
        w_hhT_sb = persist.tile([128, KC_U, G], BF16)
        w_outT_sb = persist.tile([128, KC_U, D], BF16)
        lngT_sb = persist.tile([128, KC_D], F32)
        lnbT_sb = persist.tile([128, KC_D], F32)
        binT_sb = persist.tile([128, MT_U], F32)
        bgT_sb = persist.tile([128, MT_G], F32)
        bhhnT_sb = persist.tile([128, 2 * KC_U], BF16)
        ident = persist.tile([128, 128], F32)
        eps_t = persist.tile([128, 1], F32)
        h0bf = persist.tile([128, 2 * KC_U], BF16)
        h0f = persist.tile([128, 2 * KC_U], F32)

        nc.sync.dma_start(out=w_inT_sb, in_=w_inT_ext[:, :, :])
        nc.sync.dma_start(out=w_ihT_sb, in_=w_ihT_ext[:, :, :])
        nc.sync.dma_start(out=w_hhT_sb, in_=w_hhT_ext[:, :, :])
        nc.sync.dma_start(out=w_outT_sb, in_=w_outT_ext[:, :, :])
        nc.sync.dma_start(out=lngT_sb, in_=lngT_ext[:, :])
        nc.sync.dma_start(out=lnbT_sb, in_=lnbT_ext[:, :])
        nc.sync.dma_start(out=binT_sb, in_=binT_ext[:, :])
        nc.sync.dma_start(out=bgT_sb, in_=bgT_ext[:, :])
        nc.sync.dma_start(out=bhhnT_sb, in_=bhhnT_ext[:, :])
        make_identity(nc, ident)
        nc.vector.memset(eps_t, LN_EPS)
        nc.vector.memset(h0bf, 0.0)
        nc.vector.memset(h0f, 0.0)
        # DVE "touch" of DMA-delivered tensors consumed by DVE ops, so their
        # queue sems are observed once here instead of adding waits later.
        touch = persist.tile([128, 8], F32)
        for ti, src in enumerate((lngT_sb, lnbT_sb, binT_sb, bgT_sb, bhhnT_sb)):
            nc.vector.tensor_copy(touch[:, ti : ti + 1], src[:, 0:1])
        # First PE instruction: absorb the gpsimd identity wait.
        pt0 = tr_psum.tile([128, 128], F32, tag="pt")
        nc.tensor.transpose(pt0, ident, ident)
        # broadcast b_hh n-part into gxall rows 8-11 for every step
        for c in range(KC_U):
            bh = bhhnT_sb[:, B_LOC * c : B_LOC * (c + 1)]
            bhb = bass.AP(
                tensor=bh.tensor,
                offset=bh.offset,
                ap=[bh.ap[0], [0, S], [1, B_LOC]],
            )
            nc.sync.dma_start(
                out=gxall[:, 2 * KC_U + c, :].rearrange(
                    "p (t j) -> p t j", j=B_LOC
                ),
                in_=bhb,
            )

        # ---- Phase 1: LayerNorm (natural layout) + transpose to xnT ----
        for i in range(TT):
            x_t = ln_pool.tile([128, D], F32, tag="x_t")
            nc.sync.dma_start(
                out=x_t,
                in_=x_tm[i * (128 // B_LOC) : (i + 1) * (128 // B_LOC), :, :],
            )
            stats = ln_small.tile([128, 2, 6], F32, tag="stats")
            xv = x_t.rearrange("p (h q) -> p h q", h=2)
            nc.vector.bn_stats(out=stats[:, 0, :], in_=xv[:, 0, :])
            nc.vector.bn_stats(out=stats[:, 1, :], in_=xv[:, 1, :])
            mv = ln_small.tile([128, 2], F32, tag="mv")
            nc.vector.bn_aggr(out=mv, in_=stats)
            rstd = ln_small.tile([128, 1], F32, tag="rstd")
            nc.scalar.activation(
                out=rstd, in_=mv[:, 1:2], func=AF.Sqrt, bias=eps_t, scale=1.0
            )
            nc.vector.reciprocal(out=rstd, in_=rstd)
            xc_t = ln_pool.tile([128, D], F32, tag="xc_t")
            nc.vector.tensor_scalar(
                out=xc_t,
                in0=x_t,
                scalar1=mv[:, 0:1],
                scalar2=rstd,
                op0=mybir.AluOpType.subtract,
                op1=mybir.AluOpType.mult,
            )
            for dc in range(KC_D):
                pt = tr_psum.tile([128, 128], F32, tag="pt")
                nc.tensor.transpose(pt, xc_t[:, ts(dc, 128)], ident)
                nc.vector.tensor_scalar(
                    out=xnT[:, dc, ts(i, 128)],
                    in0=pt,
                    scalar1=lngT_sb[:, dc : dc + 1],
                    scalar2=lnbT_sb[:, dc : dc + 1],
                    op0=mybir.AluOpType.mult,
                    op1=mybir.AluOpType.add,
                )
            if i == 0:
                # Dummy matmuls: let PE observe each weight tensor's DMA queue
                # sem once, keeping every real matmul at <=1 sync wait.
                pdum = mm_psum.tile([128, NCOL], F32, tag="pmm")
                for wsb in (w_inT_sb, w_ihT_sb, w_hhT_sb, w_outT_sb):
                    nc.tensor.matmul(
                        pdum[:, 0:2],
                        lhsT=wsb[:, 0, 0:128],
                        rhs=h0bf[:, 0:2],
                        start=True,
                        stop=True,
                        skip_group_check=True,
                    )

        # ---- Phase 2: in-proj  uT = w_in @ xn^T  (+ b_in) ----
        for m in range(MT_U):
            for ncol in range(NT):
                pa = mm_psum.tile([128, NCOL], F32, tag="pmm")
                for kc in range(KC_D):
                    nc.tensor.matmul(
                        pa,
                        lhsT=w_inT_sb[:, kc, ts(m, 128)],
                        rhs=xnT[:, kc, ts(ncol, NCOL)],
                        start=(kc == 0),
                        stop=(kc == KC_D - 1),
                    )
                nc.vector.tensor_scalar_add(
                    out=uT[:, m, ts(ncol, NCOL)],
                    in0=pa,
                    scalar1=binT_sb[:, m : m + 1],
                )

        # ---- Phase 3: input gates  gx^T = w_ih @ u^T (+ biases) ----
        for m in range(MT_G):
            for ncol in range(NT):
                pb = mm_psum.tile([128, NCOL], F32, tag="pmm")
                for kc in range(KC_U):
                    nc.tensor.matmul(
                        pb,
                        lhsT=w_ihT_sb[:, kc, ts(m, 128)],
                        rhs=uT[:, kc, ts(ncol, NCOL)],
                        start=(kc == 0),
                        stop=(kc == KC_U - 1),
                    )
                if m < 2 * KC_U:
                    dst = gxall[:, m, ts(ncol, NCOL)]
                else:
                    dst = gxnT[:, m - 2 * KC_U, ts(ncol, NCOL)]
                nc.vector.tensor_scalar_add(
                    out=dst, in0=pb, scalar1=bgT_sb[:, m : m + 1]
                )

        # ---- Phase 4: GRU scan (out-proj tiles interleaved into PE idle gaps) ----
        h_prev = h0f
        n_c_emitted = 0

        def emit_outproj_tile(mt):
            xr_t = c_pool.tile([128, D], F32, tag="xr_t")
            nc.sync.dma_start(
                out=xr_t,
                in_=xres_tm[mt * (128 // B_LOC) : (mt + 1) * (128 // B_LOC), :, :],
            )
            o_t = c_pool.tile([128, D], F32, tag="o_t")
            for dn in range(D // 512):
                pc = c_psum.tile([128, 512], F32, tag="pc")
                for kc in range(KC_U):
                    nc.tensor.matmul(
                        pc,
                        lhsT=yT[:, kc, ts(mt, 128)],
                        rhs=w_outT_sb[:, kc, ts(dn, 512)],
                        start=(kc == 0),
                        stop=(kc == KC_U - 1),
                    )
                nc.vector.tensor_add(o_t[:, ts(dn, 512)], pc, xr_t[:, ts(dn, 512)])
            nc.sync.dma_start(
                out=out_tm[mt * (128 // B_LOC) : (mt + 1) * (128 // B_LOC), :, :],
                in_=o_t,
            )

        for t in range(S):
            P = sc_psum.tile([128, 512], F32, tag="P")  # full bank: isolate slots
            for m in range(MT_G):
                for kc in range(KC_U):
                    if t == 0:
                        rhs = h0bf[:, ts(kc, B_LOC)]
                    else:
                        rhs = yT[:, kc, ds(B_LOC * (t - 1), B_LOC)]
                    nc.tensor.matmul(
                        P[:, ts(m, B_LOC)],
                        lhsT=w_hhT_sb[:, kc, ts(m, 128)],
                        rhs=rhs,
                        start=(kc == 0),
                        stop=(kc == KC_U - 1),
                    )

            # elementwise GRU cell, [128, (chunk, sample)] fp32 transposed layout
            tmp = ew_pool.tile([128, 2 * MT_G], F32, tag="tmp")
            nc.vector.tensor_add(
                tmp.rearrange("p (m j) -> p m j", j=B_LOC),
                P[:, 0 : 2 * MT_G].rearrange("p (m j) -> p m j", j=B_LOC),
                gxall[:, :, ds(B_LOC * t, B_LOC)],
            )
            nc.scalar.activation(
                out=tmp[:, 0 : 4 * KC_U], in_=tmp[:, 0 : 4 * KC_U], func=AF.Sigmoid
            )
            nr = ew_pool.tile([128, 2 * KC_U], F32, tag="nr")
            nc.vector.tensor_mul(
                nr, tmp[:, 0 : 2 * KC_U], tmp[:, 4 * KC_U : 6 * KC_U]
            )
            npre = ew_pool.tile([128, 2 * KC_U], F32, tag="npre")
            nc.vector.tensor_add(
                npre.rearrange("p (m j) -> p m j", j=B_LOC),
                nr.rearrange("p (m j) -> p m j", j=B_LOC),
                gxnT[:, :, ds(B_LOC * t, B_LOC)],
            )
            nc.scalar.activation(out=npre, in_=npre, func=AF.Tanh)
            d_t = ew_pool.tile([128, 2 * KC_U], F32, tag="d_t")
            nc.vector.tensor_sub(
                d_t.rearrange("p (m j) -> p m j", j=B_LOC),
                h0f.rearrange("p (m j) -> p m j", j=B_LOC)
                if t == 0
                else yT[:, :, ds(B_LOC * (t - 1), B_LOC)],
                npre.rearrange("p (m j) -> p m j", j=B_LOC),
            )
            e_t = ew_pool.tile([128, 2 * KC_U], F32, tag="e_t")
            nc.vector.tensor_mul(e_t, tmp[:, 2 * KC_U : 4 * KC_U], d_t)
            # h state lives in yT as bf16; one op writes it and the y record
            nc.vector.tensor_add(
                yT[:, :, ds(B_LOC * t, B_LOC)],
                npre.rearrange("p (m j) -> p m j", j=B_LOC),
                e_t.rearrange("p (m j) -> p m j", j=B_LOC),
            )
            # slot an out-proj tile into the PE gap once its y columns exist
            if (t + 1) % C_PER == 0:
                mt = (t + 1) // C_PER - 1
                if mt < TT:
                    emit_outproj_tile(mt)
                    n_c_emitted += 1

        for mt in range(n_c_emitted, TT):
            emit_outproj_tile(mt)

        hl = ew_pool.tile([128, 2 * KC_U], F32, tag="hl")
        nc.vector.tensor_copy(
            hl.rearrange("p (m j) -> p m j", j=B_LOC),
            yT[:, :, ds(B_LOC * (S - 1), B_LOC)],
        )
        nc.sync.dma_start(out=hlastT_ext[:, :], in_=hl)
        if debug:
            nc.sync.dma_start(out=xnT_d[:, :, :], in_=xnT)
            nc.sync.dma_start(out=uT_d[:, :, :], in_=uT)
            nc.sync.dma_start(out=gxrz_d[:, :, :], in_=gxall[:, : 2 * KC_U, :])
            nc.sync.dma_start(out=gxn_d[:, :, :], in_=gxnT)
            nc.sync.dma_start(out=yT_d[:, :, :], in_=yT)

    _legalize_waits(nc)
    return nc


# Sequencer-class instructions accept many embedded sync waits; engine-datapath
# instructions accept only ONE (walrus "Too many sync wait commands"). Move
# excess waits onto a preceding same-engine sequencer NOP.
_SEQ_CLASS = {
    "InstNoOp", "InstDrain", "InstEventSemaphore", "InstUnconditionalBranch",
    "InstConditionalBranch", "InstRegisterMove", "InstCall", "InstISA",
    "InstRegisterAlu", "InstCompareAndBranch",
}


def _legalize_waits(nc):
    n_split = 0
    for fn in nc.m.functions:
        for blk in fn.blocks:
            il = blk.instructions
            idx = 0
            while idx < len(il):
                inst = il[idx]
                si = inst.sync_info
                if si is not None and si.on_wait and len(si.on_wait) > 1:
                    waits = list(si.on_wait)
                    keep = waits[-1:]
                    rest = waits[:-1]
                    nops = []
                    for w in rest:
                        nop = mybir.InstNoOp(
                            name=f"legw-{n_split}",
                            engine=inst.engine,
                            ins=[],
                            outs=[],
                        )
                        nop.sync_info = mybir.SyncInfo(on_wait=[w], on_update=[])
                        nops.append(nop)
                        n_split += 1
                    inst.sync_info = mybir.SyncInfo(
                        on_wait=keep, on_update=list(si.on_update or [])
                    )
                    for j, nop in enumerate(nops):
                        il.insert(idx + j, nop)
                    idx += len(nops)
                idx += 1
    return n_split


def _prep_shared_inputs(ln_gamma, ln_beta, w_in, b_in, w_ih, w_hh, b_ih, b_hh, w_out):
    """Host-side weight layout transforms (shared across cores)."""
    bf = ml_dtypes.bfloat16

    def chunked_T(w):  # [O, I] -> [128, I//128, O] (transposed, k-chunked)
        o, i = w.shape
        return np.ascontiguousarray(
            w.T.reshape(i // 128, 128, o).transpose(1, 0, 2)
        )

    lngT = np.ascontiguousarray(ln_gamma.reshape(KC_D, 128).T).astype(np.float32)
    lnbT = np.ascontiguousarray(ln_beta.reshape(KC_D, 128).T).astype(np.float32)
    w_inT = chunked_T(w_in).astype(bf)
    binT = np.ascontiguousarray(b_in.reshape(MT_U, 128).T).astype(np.float32)
    w_ihT = chunked_T(w_ih).astype(bf)
    # r,z gate tiles carry b_ih+b_hh; n tiles carry only b_ih (b_hh_n applied in-scan)
    bg = (b_ih + b_hh).astype(np.float32).copy()
    bg[2 * U :] = b_ih[2 * U :]
    bgT = np.ascontiguousarray(bg.reshape(MT_G, 128).T).astype(np.float32)
    w_hhT = chunked_T(w_hh).astype(bf)
    bhhn = b_hh[2 * U :].astype(np.float32).reshape(KC_U, 128).T  # [128, 4]
    bhhnT = np.ascontiguousarray(
        np.repeat(bhhn[:, :, None], B_LOC, axis=2).reshape(128, KC_U * B_LOC)
    ).astype(bf)
    w_outT = chunked_T(w_out).astype(bf)
    return dict(
        lngT=lngT, lnbT=lnbT, w_inT=w_inT, binT=binT, w_ihT=w_ihT,
        bgT=bgT, w_hhT=w_hhT, bhhnT=bhhnT, w_outT=w_outT,
    )


_BUILD_CACHE = {}


def _get_program(S):
    if S not in _BUILD_CACHE:
        _BUILD_CACHE[S] = build_program(S)
    return _BUILD_CACHE[S]


LAST_EXEC_TIME_NS = None


def run(x, ln_gamma, ln_beta, w_in, b_in, w_ih, w_hh, b_ih, b_hh, w_out, b_out,
        trace=False, S=S_FULL):
    global LAST_EXEC_TIME_NS
    x = np.asarray(x, dtype=np.float32)
    b_full = x.shape[0]
    n_cores = b_full // B_LOC
    shared = _prep_shared_inputs(
        np.asarray(ln_gamma, np.float32), np.asarray(ln_beta, np.float32),
        np.asarray(w_in, np.float32), np.asarray(b_in, np.float32),
        np.asarray(w_ih, np.float32), np.asarray(w_hh, np.float32),
        np.asarray(b_ih, np.float32), np.asarray(b_hh, np.float32),
        np.asarray(w_out, np.float32),
    )
    xres = x + np.asarray(b_out, np.float32)[None, None, :]

    nc = _get_program(S)
    in_maps = []
    for k in range(n_cores):
        m = dict(shared)
        m["x"] = np.ascontiguousarray(x[k * B_LOC : (k + 1) * B_LOC])
        m["xres"] = np.ascontiguousarray(xres[k * B_LOC : (k + 1) * B_LOC])
        in_maps.append(m)

    try:
        res = run_bass_kernel_spmd(nc, in_maps, list(range(n_cores)), trace=trace)
    except ModuleNotFoundError:
        res = run_bass_kernel_spmd(nc, in_maps, list(range(n_cores)), trace=False)
    LAST_EXEC_TIME_NS = res.exec_time_ns

    out = np.empty((b_full, S, D), np.float32)
    h_last = np.empty((b_full, U), np.float32)
    for k in range(n_cores):
        out[k * B_LOC : (k + 1) * B_LOC] = res.results[k]["out"]
        hlT = res.results[k]["hlastT"].reshape(128, KC_U, B_LOC)
        h_last[k * B_LOC : (k + 1) * B_LOC] = (
            hlT.transpose(2, 1, 0).reshape(B_LOC, U)
        )
    return out, h_last


def kernel(x, ln_gamma, ln_beta, w_in, b_in, w_ih, w_hh, b_ih, b_hh, w_out, b_out):
    return run(
        x, ln_gamma, ln_beta, w_in, b_in, w_ih, w_hh, b_ih, b_hh, w_out, b_out,
        trace=False, S=S_FULL,
    )
